# revision 5
# baseline (speedup 1.0000x reference)
"""Distributed Trainium2 attention kernel (8 NeuronCores, head tensor-parallel).

Reference semantics (T=4096, D=2048, H=16, DH=128):
  qkv = bf16(x @ W_qkv); q,k,v per head; RoPE(split-half) on q,k;
  mask = ((m_q & m_k) | eye) & causal; softmax(q k^T / sqrt(DH) masked);
  out = bf16((probs @ v) @ W_out)

Sharding: head tensor-parallel. Core c owns heads (2c, 2c+1): W_qkv column
shard, W_out row shard, full x (replicated, passed pre-transposed).
Each core computes its heads' SDPA, its out-projection partial, then a
chunked ReduceScatter sums partials; core c emits output rows
[chunk*1024 + c*128 : +128) for each of the 4 chunks. Host reassembles.

Device-side layout choices:
  - x passed as xT [D, T] so the D contraction dim is the partition dim.
  - q,k computed weight-stationary -> born transposed [DH, T] (what the
    scores matmul needs); v transposed back to natural [T, DH] via PE.
  - RoPE: partition-rotate by 64 via a permutation-matrix matmul on PE,
    sign folded into a host-precomputed ssinT table; combine on DVE.
  - mask folded into a host-precomputed additive bias [T, T] bf16
    streamed from DRAM (0 where allowed, -1e9 where masked).
  - softmax without max-subtraction (scores are O(5) here); exp on ACT
    with accum_out producing the denominator in the same pass.
  - probs scaled by 1/denom, transposed 128x128 on PE; PV accumulates
    o^T [DH, T] directly, which is the lhsT layout out-proj wants.
"""

import os
import sys

import numpy as np

sys.path.insert(0, "/opt/trn_rl_repo")

import ml_dtypes

BF16 = ml_dtypes.bfloat16

# problem constants (hardcoded per harness contract)
T, D, H, DH = 4096, 2048, 16, 128
N_CORES = 8
ROPE_BASE = 10000.0


def build_nc(
    t=T,
    d=D,
    n_cores=N_CORES,
    hl=H // N_CORES,  # heads per core
    kch=512,  # scores k-chunk (free dim of scores matmul)
    tch=512,  # qkv t-chunk
    rs_chunks=4,  # reduce-scatter chunks
):
    import concourse.bass as bass
    import concourse.mybir as mybir
    import concourse.tile as tile
    from concourse import bacc
    from concourse.masks import make_identity

    f32 = mybir.dt.float32
    bf16 = mybir.dt.bfloat16

    P = 128
    kd = d // P  # contraction chunks for qkv
    qb_n = t // P  # q-blocks of 128 rows
    nt = t // tch  # t-chunks in qkv phase
    jl = hl * P  # local out-proj contraction width
    qb_per_chunk = qb_n // rs_chunks
    t_out = t // n_cores  # output rows per core
    scale = 1.0 / np.sqrt(DH)

    nc = bacc.Bacc(
        "TRN2", target_bir_lowering=False, debug=False, num_devices=n_cores
    )

    xT = nc.dram_tensor("xT", [d, t], bf16, kind="ExternalInput").ap()
    wqkv = nc.dram_tensor("wqkv", [d, 3 * jl], bf16, kind="ExternalInput").ap()
    wout_d = nc.dram_tensor("wout", [jl, d], bf16, kind="ExternalInput").ap()
    cosT_d = nc.dram_tensor("cosT", [P, t], f32, kind="ExternalInput").ap()
    ssinT_d = nc.dram_tensor("ssinT", [P, t], f32, kind="ExternalInput").ap()
    rot_d = nc.dram_tensor("rot", [P, P], bf16, kind="ExternalInput").ap()
    bias_d = nc.dram_tensor("bias", [t, t], bf16, kind="ExternalInput").ap()
    out_d = nc.dram_tensor("out", [t_out, d], bf16, kind="ExternalOutput").ap()

    with tile.TileContext(nc) as tc:
        with tc.tile_pool(name="persist", bufs=1) as persist:
            # persistent SBUF tensors
            ident = persist.tile([P, P], bf16, name="ident")
            make_identity(nc, ident)
            rot_sb = persist.tile([P, P], bf16, name="rot_sb")
            nc.sync.dma_start(rot_sb, rot_d)
            wq_sb = persist.tile([P, kd, 3 * hl, P], bf16, name="wq_sb")
            nc.sync.dma_start(
                wq_sb, wqkv.rearrange("(kd p) (c j) -> p kd c j", p=P, j=P)
            )
            wout_sb = persist.tile([P, hl, d], bf16, name="wout_sb")
            nc.sync.dma_start(wout_sb, wout_d.rearrange("(h p) x -> p h x", p=P))

            # per-head persistent activations
            qT = [persist.tile([P, t], bf16, name=f"qT{h}") for h in range(hl)]
            kT = [persist.tile([P, t], bf16, name=f"kT{h}") for h in range(hl)]
            vT = [persist.tile([P, t], bf16, name=f"vT{h}") for h in range(hl)]
            v_nat = [
                persist.tile([P, qb_n, P], bf16, name=f"vnat{h}") for h in range(hl)
            ]
            oT = [persist.tile([P, t], bf16, name=f"oT{h}") for h in range(hl)]

            # ---------------- phase 1: qkv + rope + v transpose ----------
            with (
                tc.tile_pool(name="ph1", bufs=2) as ph1,
                tc.tile_pool(name="cs", bufs=1) as cspool,
                tc.tile_pool(name="ps_qkv", bufs=1, space="PSUM") as ps_qkv,
                tc.tile_pool(name="ps_aux", bufs=2, space="PSUM") as ps_aux,
            ):
                cosT_sb = cspool.tile([P, t], f32, name="cosT_sb")
                nc.sync.dma_start(cosT_sb, cosT_d)
                ssinT_sb = cspool.tile([P, t], f32, name="ssinT_sb")
                nc.sync.dma_start(ssinT_sb, ssinT_d)

                for tc_i in range(nt):
                    tsl = slice(tc_i * tch, (tc_i + 1) * tch)
                    xt = ph1.tile([P, kd, tch], bf16, tag="xt")
                    nc.sync.dma_start(
                        xt, xT.rearrange("(kd p) x -> p kd x", p=P)[:, :, tsl]
                    )
                    for c in range(3 * hl):  # q0,q1,k0,k1,v0,v1
                        ps = ps_qkv.tile([P, tch], mybir.dt.float32, tag=f"ps{c}")
                        for k in range(kd):
                            nc.tensor.matmul(
                                ps,
                                lhsT=wq_sb[:, k, c],
                                rhs=xt[:, k],
                                start=(k == 0),
                                stop=(k == kd - 1),
                            )
                        if c < 2 * hl:  # q or k: cast, rotate, rope-combine
                            dst = qT[c] if c < hl else kT[c - hl]
                            qbf = ph1.tile([P, tch], bf16, tag="qbf")
                            nc.scalar.copy(qbf, ps)
                            shift = ps_aux.tile(
                                [P, tch], mybir.dt.float32, tag="aux"
                            )
                            nc.tensor.matmul(
                                shift, lhsT=rot_sb, rhs=qbf, start=True, stop=True
                            )
                            t1 = ph1.tile([P, tch], f32, tag="t1")
                            nc.vector.tensor_tensor(
                                t1, qbf, cosT_sb[:, tsl], mybir.AluOpType.mult
                            )
                            t2 = ph1.tile([P, tch], f32, tag="t2")
                            nc.vector.tensor_tensor(
                                t2, shift, ssinT_sb[:, tsl], mybir.AluOpType.mult
                            )
                            nc.vector.tensor_tensor(
                                dst[:, tsl], t1, t2, mybir.AluOpType.add
                            )
                        else:  # v: just cast
                            nc.scalar.copy(vT[c - 2 * hl][:, tsl], ps)

                # v: [DH, T] -> natural [T-block, DH] tiles
                for h in range(hl):
                    for b in range(qb_n):
                        pst = ps_aux.tile([P, P], bf16, tag="aux")
                        nc.tensor.transpose(
                            pst, vT[h][:, b * P : (b + 1) * P], ident
                        )
                        nc.scalar.copy(v_nat[h][:, b], pst)

            # ---------------- phase 2: SDPA + out-proj + RS --------------
            with (
                tc.tile_pool(name="ph2", bufs=3) as ph2,
                tc.tile_pool(name="pr", bufs=2) as prpool,
                tc.tile_pool(name="dram", bufs=1, space="DRAM") as dram,
                tc.tile_pool(name="ps_s", bufs=2, space="PSUM") as ps_s,
                tc.tile_pool(name="ps_tr", bufs=2, space="PSUM") as ps_tr,
                tc.tile_pool(name="ps_o", bufs=2, space="PSUM") as ps_o,
                tc.tile_pool(name="ps_out", bufs=2, space="PSUM") as ps_out,
            ):
                rs_in = [
                    dram.tile([qb_per_chunk * P * n_cores // n_cores, d], bf16,
                              name=f"rs_in{ci}")
                    for ci in range(rs_chunks)
                ]
                rs_out = [
                    dram.tile([qb_per_chunk * P // n_cores, d], bf16,
                              name=f"rs_out{ci}")
                    for ci in range(rs_chunks)
                ]

                for qb in range(qb_n):
                    nkc = (qb * P + P + kch - 1) // kch  # causal k-chunks
                    nsk = qb + 1  # causal 128-blocks
                    qsl = slice(qb * P, (qb + 1) * P)
                    for h in range(hl):
                        probs = prpool.tile([P, qb_n * P], bf16, tag="probs")
                        denom = ph2.tile([P, 32], f32, tag="denom")
                        for kc in range(nkc):
                            ksl = slice(kc * kch, (kc + 1) * kch)
                            pss = ps_s.tile([P, kch], f32, tag="scores")
                            nc.tensor.matmul(
                                pss,
                                lhsT=qT[h][:, qsl],
                                rhs=kT[h][:, ksl],
                                start=True,
                                stop=True,
                            )
                            bt = ph2.tile([P, kch], bf16, tag="bias")
                            nc.sync.dma_start(bt, bias_d[qsl, ksl])
                            nc.vector.tensor_tensor(
                                pss, pss, bt, mybir.AluOpType.add
                            )
                            nc.scalar.activation(
                                probs[:, ksl],
                                pss,
                                mybir.ActivationFunctionType.Exp,
                                scale=float(scale),
                                accum_out=denom[:, kc : kc + 1],
                            )
                        dsum = ph2.tile([P, 1], f32, tag="dsum")
                        nc.vector.reduce_sum(
                            dsum, denom[:, :nkc], axis=mybir.AxisListType.X
                        )
                        inv = ph2.tile([P, 1], f32, tag="inv")
                        nc.vector.reciprocal(inv, dsum)

                        pso = ps_o.tile([P, P], f32, tag="oT")
                        for sk in range(nsk):
                            ssl = slice(sk * P, (sk + 1) * P)
                            pscl = ph2.tile([P, P], bf16, tag="pscl")
                            nc.vector.tensor_scalar_mul(pscl, probs[:, ssl], inv)
                            pst = ps_tr.tile([P, P], bf16, tag="ptr")
                            nc.tensor.transpose(pst, pscl, ident)
                            prT = ph2.tile([P, P], bf16, tag="prT")
                            nc.scalar.copy(prT, pst)
                            nc.tensor.matmul(
                                pso,
                                lhsT=v_nat[h][:, sk],
                                rhs=prT,
                                start=(sk == 0),
                                stop=(sk == nsk - 1),
                            )
                        nc.scalar.copy(oT[h][:, qsl], pso)

                    # out-projection for this q-block's rows
                    partial = ph2.tile([P, d], bf16, tag="partial")
                    for ntile in range(d // 512):
                        nsl = slice(ntile * 512, (ntile + 1) * 512)
                        pso2 = ps_out.tile([P, 512], f32, tag="outp")
                        for h in range(hl):
                            nc.tensor.matmul(
                                pso2,
                                lhsT=oT[h][:, qsl],
                                rhs=wout_sb[:, h, nsl],
                                start=(h == 0),
                                stop=(h == hl - 1),
                            )
                        nc.scalar.copy(partial[:, nsl], pso2)
                    ci = qb // qb_per_chunk
                    ri = qb % qb_per_chunk
                    nc.sync.dma_start(
                        rs_in[ci][ri * P : (ri + 1) * P, :], partial
                    )
                    if ri == qb_per_chunk - 1:
                        nc.gpsimd.collective_compute(
                            "ReduceScatter",
                            mybir.AluOpType.add,
                            replica_groups=[list(range(n_cores))],
                            ins=[rs_in[ci].opt()],
                            outs=[rs_out[ci].opt()],
                        )
                        rows = qb_per_chunk * P // n_cores
                        nc.sync.dma_start(
                            out_d[ci * rows : (ci + 1) * rows, :], rs_out[ci]
                        )

    nc.compile()
    return nc


def prepare_in_maps(x, W_qkv, W_out, cos, sin, mask, n_cores=N_CORES, hl=H // N_CORES):
    """Host-side sharding. Returns list of per-core input dicts."""
    t, d = x.shape
    x = np.asarray(x, dtype=BF16)
    W_qkv = np.asarray(W_qkv, dtype=BF16)
    W_out = np.asarray(W_out, dtype=BF16)
    cos = np.asarray(cos, dtype=np.float32)
    sin = np.asarray(sin, dtype=np.float32)
    m = np.asarray(mask, dtype=bool)

    xT = np.ascontiguousarray(x.T)
    cosT = np.ascontiguousarray(cos.T)
    sign = np.where(np.arange(DH) < DH // 2, -1.0, 1.0).astype(np.float32)
    ssinT = np.ascontiguousarray(sin.T * sign[:, None])
    rot = np.zeros((DH, DH), dtype=BF16)
    rot[(np.arange(DH) + DH // 2) % DH, np.arange(DH)] = 1

    allowed = ((m[:, None] & m[None, :]) | np.eye(t, dtype=bool)) & np.tril(
        np.ones((t, t), dtype=bool)
    )
    bias = np.where(allowed, np.float32(0.0), np.float32(-1e9)).astype(BF16)

    n_heads = W_qkv.shape[1] // 3 // DH
    in_maps = []
    for c in range(n_cores):
        hs = [c * hl + i for i in range(hl)]
        cols = [W_qkv[:, (s * n_heads + h) * DH : (s * n_heads + h) * DH + DH]
                for s in range(3) for h in hs]
        wqkv_c = np.ascontiguousarray(np.concatenate(cols, axis=1))
        wout_c = np.ascontiguousarray(
            W_out[hs[0] * DH : (hs[-1] + 1) * DH, :]
        )
        in_maps.append(
            {
                "xT": xT,
                "wqkv": wqkv_c,
                "wout": wout_c,
                "cosT": cosT,
                "ssinT": ssinT,
                "rot": rot,
                "bias": bias,
            }
        )
    return in_maps


_CACHED_NC = None


def assemble(results, t=T, d=D, n_cores=N_CORES, rs_chunks=4):
    """Reassemble per-core ReduceScatter slices into the full output."""
    rows_per_chunk = t // rs_chunks
    rows_per_core = rows_per_chunk // n_cores
    out = np.empty((t, d), dtype=BF16)
    for c in range(n_cores):
        oc = np.asarray(results[c]["out"])
        if oc.dtype != BF16:
            oc = oc.view(BF16)
        for ci in range(rs_chunks):
            lo = ci * rows_per_chunk + c * rows_per_core
            out[lo : lo + rows_per_core] = oc[
                ci * rows_per_core : (ci + 1) * rows_per_core
            ]
    return out


def kernel(x, W_qkv, W_out, cos, sin, mask):
    """Full inputs in, full output out. Shards across 8 NeuronCores."""
    global _CACHED_NC
    from concourse import bass_utils

    if _CACHED_NC is None:
        _CACHED_NC = build_nc()
    nc = _CACHED_NC

    in_maps = prepare_in_maps(x, W_qkv, W_out, cos, sin, mask)
    res = bass_utils.run_bass_kernel_spmd(
        nc, in_maps, core_ids=list(range(N_CORES))
    )
    return assemble(res.results)


# revision 20
# speedup vs baseline: 1.3255x; 1.3255x over previous
"""Distributed Trainium2 attention kernel (8 NeuronCores, head tensor-parallel).

Reference semantics (T=4096, D=2048, H=16, DH=128):
  qkv = bf16(x @ W_qkv); q,k,v per head; RoPE(split-half) on q,k;
  mask = ((m_q & m_k) | eye) & causal; softmax(q k^T / sqrt(DH) masked);
  out = bf16((probs @ v) @ W_out)

Sharding: head tensor-parallel. Core c owns heads (2c, 2c+1): W_qkv column
shard, W_out row shard, full x (replicated, passed pre-transposed).
Each core computes its heads' SDPA, its out-projection partial, then a
chunked ReduceScatter sums partials; core c emits output rows
[chunk*1024 + c*128 : +128) for each of the 4 chunks. Host reassembles.

Device-side layout choices:
  - x passed as xT [D, T] so the D contraction dim is the partition dim.
  - q,k computed weight-stationary -> born transposed [DH, T] (what the
    scores matmul needs); v transposed back to natural [T, DH] via PE.
  - RoPE: partition-rotate by 64 via a permutation-matrix matmul on PE,
    sign folded into a host-precomputed ssinT table; combine on DVE.
  - mask folded into a host-precomputed additive bias [T, T] bf16
    streamed from DRAM (0 where allowed, -1e9 where masked).
  - softmax without max-subtraction (scores are O(5) here); exp on ACT
    with accum_out producing the denominator in the same pass.
  - probs scaled by 1/denom, transposed 128x128 on PE; PV accumulates
    o^T [DH, T] directly, which is the lhsT layout out-proj wants.
"""

import os
import sys

import numpy as np

sys.path.insert(0, "/opt/trn_rl_repo")

import ml_dtypes

BF16 = ml_dtypes.bfloat16

# problem constants (hardcoded per harness contract)
T, D, H, DH = 4096, 2048, 16, 128
N_CORES = 8
ROPE_BASE = 10000.0


def build_nc(
    t=T,
    d=D,
    n_cores=N_CORES,
    hl=H // N_CORES,  # heads per core
    kch=512,  # scores k-chunk (free dim of scores matmul)
    tch=512,  # qkv t-chunk
    rs_chunks=4,  # reduce-scatter chunks
):
    import concourse.bass as bass
    import concourse.mybir as mybir
    import concourse.tile as tile
    from concourse import bacc
    from concourse.masks import make_identity

    f32 = mybir.dt.float32
    bf16 = mybir.dt.bfloat16

    P = 128
    kd = d // P  # contraction chunks for qkv
    qb_n = t // P  # q-blocks of 128 rows
    nt = t // tch  # t-chunks in qkv phase
    jl = hl * P  # local out-proj contraction width
    qb_per_chunk = qb_n // rs_chunks
    t_out = t // n_cores  # output rows per core
    scale = 1.0 / np.sqrt(DH)

    nc = bacc.Bacc(
        "TRN2", target_bir_lowering=False, debug=False, num_devices=n_cores
    )

    xT = nc.dram_tensor("xT", [d, t], bf16, kind="ExternalInput").ap()
    wqkv = nc.dram_tensor("wqkv", [d, 3 * jl], bf16, kind="ExternalInput").ap()
    wout_d = nc.dram_tensor("wout", [jl, d], bf16, kind="ExternalInput").ap()
    cosT_d = nc.dram_tensor("cosT", [P, t], f32, kind="ExternalInput").ap()
    ssinT_d = nc.dram_tensor("ssinT", [P, t], f32, kind="ExternalInput").ap()
    rot_d = nc.dram_tensor("rot", [P, P], bf16, kind="ExternalInput").ap()
    # colmask: mask[k] as 0/1 bf16, pre-broadcast to all 128 partitions
    colmask_d = nc.dram_tensor("colmask", [P, t], bf16, kind="ExternalInput").ap()
    # rqT[p, qb] = 0 if mask[qb*128+p] else -1e9 (folded into exp bias)
    rqT_d = nc.dram_tensor("rqT", [P, qb_n], f32, kind="ExternalInput").ap()
    # dvalT[p, qb] = 1 - mask[qb*128+p] (diagonal restore for masked rows)
    dvalT_d = nc.dram_tensor("dvalT", [P, qb_n], f32, kind="ExternalInput").ap()
    # cmask[p, v, j] = 1 if v*128 + p >= j else 0 (causal diag-chunk variants)
    cmask_d = nc.dram_tensor("cmask", [P, 4, kch], bf16, kind="ExternalInput").ap()
    out_d = nc.dram_tensor("out", [t_out, d], bf16, kind="ExternalOutput").ap()

    with tile.TileContext(nc) as tc:
        with tc.tile_pool(name="persist", bufs=1) as persist:
            # persistent SBUF tensors
            ident = persist.tile([P, P], bf16, name="ident")
            make_identity(nc, ident)
            rot_sb = persist.tile([P, P], bf16, name="rot_sb")
            nc.sync.dma_start(rot_sb, rot_d)
            wq_sb = persist.tile([P, kd, 3 * hl, P], bf16, name="wq_sb")
            nc.sync.dma_start(
                wq_sb, wqkv.rearrange("(kd p) (c j) -> p kd c j", p=P, j=P)
            )
            wout_sb = persist.tile([P, hl, d], bf16, name="wout_sb")
            nc.sync.dma_start(wout_sb, wout_d.rearrange("(h p) x -> p h x", p=P))

            # per-head persistent activations
            qT = [persist.tile([P, t], bf16, name=f"qT{h}") for h in range(hl)]
            kT = [persist.tile([P, t], bf16, name=f"kT{h}") for h in range(hl)]
            vT = [persist.tile([P, t], bf16, name=f"vT{h}") for h in range(hl)]
            v_nat = [
                persist.tile([P, qb_n, P], bf16, name=f"vnat{h}") for h in range(hl)
            ]
            oT = [persist.tile([P, t], bf16, name=f"oT{h}") for h in range(hl)]

            # ---------------- phase 1: qkv + rope + v transpose ----------
            with (
                tc.tile_pool(name="ph1", bufs=2) as ph1,
                tc.tile_pool(name="cs", bufs=1) as cspool,
                tc.tile_pool(name="ps_qkv", bufs=1, space="PSUM") as ps_qkv,
                tc.tile_pool(name="ps_aux", bufs=2, space="PSUM") as ps_aux,
            ):
                cosT_sb = cspool.tile([P, t], f32, name="cosT_sb")
                nc.sync.dma_start(cosT_sb, cosT_d)
                ssinT_sb = cspool.tile([P, t], f32, name="ssinT_sb")
                nc.sync.dma_start(ssinT_sb, ssinT_d)

                for tc_i in range(nt):
                    tsl = slice(tc_i * tch, (tc_i + 1) * tch)
                    xt = ph1.tile([P, kd, tch], bf16, tag="xt")
                    nc.sync.dma_start(
                        xt, xT.rearrange("(kd p) x -> p kd x", p=P)[:, :, tsl]
                    )
                    for c in range(3 * hl):  # q0,q1,k0,k1,v0,v1
                        ps = ps_qkv.tile([P, tch], mybir.dt.float32, tag=f"ps{c}")
                        for k in range(kd):
                            nc.tensor.matmul(
                                ps,
                                lhsT=wq_sb[:, k, c],
                                rhs=xt[:, k],
                                start=(k == 0),
                                stop=(k == kd - 1),
                            )
                        if c < 2 * hl:  # q or k: cast, rotate, rope-combine
                            dst = qT[c] if c < hl else kT[c - hl]
                            qbf = ph1.tile([P, tch], bf16, tag="qbf")
                            nc.scalar.copy(qbf, ps)
                            shift = ps_aux.tile(
                                [P, tch], mybir.dt.float32, tag="aux"
                            )
                            nc.tensor.matmul(
                                shift, lhsT=rot_sb, rhs=qbf, start=True, stop=True
                            )
                            t1 = ph1.tile([P, tch], f32, tag="t1")
                            nc.vector.tensor_tensor(
                                t1, qbf, cosT_sb[:, tsl], mybir.AluOpType.mult
                            )
                            t2 = ph1.tile([P, tch], f32, tag="t2")
                            nc.vector.tensor_tensor(
                                t2, shift, ssinT_sb[:, tsl], mybir.AluOpType.mult
                            )
                            nc.vector.tensor_tensor(
                                dst[:, tsl], t1, t2, mybir.AluOpType.add
                            )
                        else:  # v: just cast
                            nc.scalar.copy(vT[c - 2 * hl][:, tsl], ps)

                # v: [DH, T] -> natural [T-block, DH] tiles
                for h in range(hl):
                    for b in range(qb_n):
                        pst = ps_aux.tile([P, P], bf16, tag="aux")
                        nc.tensor.transpose(
                            pst, vT[h][:, b * P : (b + 1) * P], ident
                        )
                        nc.scalar.copy(v_nat[h][:, b], pst)

            # ---------------- phase 2: SDPA + out-proj + RS --------------
            scw = min(2048, t)  # scores psum tile width (4 banks)
            with (
                tc.tile_pool(name="ph2", bufs=3) as ph2,
                tc.tile_pool(name="pr", bufs=2) as prpool,
                tc.tile_pool(name="msk", bufs=1) as mskpool,
                tc.tile_pool(name="dram", bufs=1, space="DRAM") as dram,
                tc.tile_pool(name="ps_s", bufs=1, space="PSUM") as ps_s,
                tc.tile_pool(name="ps_tr", bufs=2, space="PSUM") as ps_tr,
                tc.tile_pool(name="ps_o", bufs=1, space="PSUM") as ps_o,
                tc.tile_pool(name="ps_out", bufs=1, space="PSUM") as ps_out,
            ):
                colmask_sb = mskpool.tile([P, t], bf16, name="colmask_sb")
                nc.sync.dma_start(colmask_sb, colmask_d)
                rqT_sb = mskpool.tile([P, qb_n], f32, name="rqT_sb")
                nc.sync.dma_start(rqT_sb, rqT_d)
                dvalT_sb = mskpool.tile([P, qb_n], f32, name="dvalT_sb")
                nc.sync.dma_start(dvalT_sb, dvalT_d)
                cmask_sb = mskpool.tile([P, 4, kch], bf16, name="cmask_sb")
                nc.sync.dma_start(cmask_sb, cmask_d)
                rs_in = [
                    dram.tile([qb_per_chunk * P * n_cores // n_cores, d], bf16,
                              name=f"rs_in{ci}")
                    for ci in range(rs_chunks)
                ]
                rs_out = [
                    dram.tile([qb_per_chunk * P // n_cores, d], bf16,
                              name=f"rs_out{ci}")
                    for ci in range(rs_chunks)
                ]

                for qb in range(qb_n):
                    nkc = (qb * P + P + kch - 1) // kch  # causal 512-subchunks
                    w = nkc * kch  # computed scores width
                    nsk = qb + 1  # causal 128-blocks
                    qsl = slice(qb * P, (qb + 1) * P)
                    dsl = slice(w - kch, w)  # diagonal 512-subchunk
                    for h in range(hl):
                        probs = prpool.tile([P, qb_n * P], bf16, tag="probs")
                        # scores + exp in <=scw segments (psum capacity)
                        for seg in range(0, w, scw):
                            sw = min(scw, w - seg)
                            pss = ps_s.tile([P, scw], f32, tag="scores")
                            for kc in range(0, sw, kch):
                                nc.tensor.matmul(
                                    pss[:, kc : kc + kch],
                                    lhsT=qT[h][:, qsl],
                                    rhs=kT[h][:, seg + kc : seg + kc + kch],
                                    start=True,
                                    stop=True,
                                )
                            # exp over the segment; row-mask folded into the
                            # bias (masked rows -> exp ~ 0)
                            nc.scalar.activation(
                                probs[:, seg : seg + sw],
                                pss[:, :sw],
                                mybir.ActivationFunctionType.Exp,
                                scale=float(scale),
                                bias=rqT_sb[:, qb : qb + 1],
                            )
                        # padding col-mask over the whole causal width (at the
                        # diagonal colmask[q] == m[q], consistent with the row
                        # bias: unmasked rows keep their diag, masked rows got
                        # exp~0 anyway)
                        nc.vector.tensor_tensor(
                            probs[:, :w],
                            probs[:, :w],
                            colmask_sb[:, :w],
                            mybir.AluOpType.mult,
                        )
                        # causal zero within the diag subchunk (precomputed
                        # 0/1 pattern; only 4 variants exist)
                        nc.vector.tensor_tensor(
                            probs[:, dsl],
                            probs[:, dsl],
                            cmask_sb[:, qb % 4, :],
                            mybir.AluOpType.mult,
                        )
                        # eye restore for masked rows: probs[p,p] += 1-m[q]
                        # (softmax over {self} == 1)
                        eyed = ph2.tile([P, P], bf16, tag="eyed")
                        nc.vector.tensor_scalar_mul(
                            eyed, ident, dvalT_sb[:, qb : qb + 1]
                        )
                        nc.vector.tensor_tensor(
                            probs[:, qsl], probs[:, qsl], eyed,
                            mybir.AluOpType.add,
                        )
                        # denominator over the fully-masked probs
                        dsum = ph2.tile([P, 1], f32, tag="dsum")
                        nc.vector.reduce_sum(
                            dsum, probs[:, :w], axis=mybir.AxisListType.X
                        )
                        inv = ph2.tile([P, 1], f32, tag="inv")
                        nc.vector.reciprocal(inv, dsum)
                        nc.vector.tensor_scalar_mul(
                            probs[:, :w], probs[:, :w], inv
                        )

                        pso = ps_o.tile([P, P], f32, tag="oT")
                        ngrp = (nsk + 3) // 4
                        for g in range(ngrp):
                            lo = g * 4
                            hi = min(lo + 4, nsk)
                            pst = ps_tr.tile([P, 512], bf16, tag="ptr")
                            for i in range(hi - lo):
                                ssl = slice((lo + i) * P, (lo + i + 1) * P)
                                nc.tensor.transpose(
                                    pst[:, i * P : (i + 1) * P],
                                    probs[:, ssl],
                                    ident,
                                )
                            prT = ph2.tile([P, 512], bf16, tag="prT")
                            nc.vector.tensor_copy(
                                prT[:, : (hi - lo) * P], pst[:, : (hi - lo) * P]
                            )
                            for i in range(hi - lo):
                                nc.tensor.matmul(
                                    pso,
                                    lhsT=v_nat[h][:, lo + i],
                                    rhs=prT[:, i * P : (i + 1) * P],
                                    start=(lo + i == 0),
                                    stop=(lo + i == nsk - 1),
                                )
                        nc.vector.tensor_copy(oT[h][:, qsl], pso)

                    # out-projection for this q-block's rows
                    partial = ph2.tile([P, d], bf16, tag="partial")
                    for ntile in range(d // 512):
                        nsl = slice(ntile * 512, (ntile + 1) * 512)
                        pso2 = ps_out.tile([P, 512], f32, tag="outp")
                        for h in range(hl):
                            nc.tensor.matmul(
                                pso2,
                                lhsT=oT[h][:, qsl],
                                rhs=wout_sb[:, h, nsl],
                                start=(h == 0),
                                stop=(h == hl - 1),
                            )
                        nc.vector.tensor_copy(partial[:, nsl], pso2)
                    ci = qb // qb_per_chunk
                    ri = qb % qb_per_chunk
                    nc.sync.dma_start(
                        rs_in[ci][ri * P : (ri + 1) * P, :], partial
                    )
                    if ri == qb_per_chunk - 1:
                        nc.gpsimd.collective_compute(
                            "ReduceScatter",
                            mybir.AluOpType.add,
                            replica_groups=[list(range(n_cores))],
                            ins=[rs_in[ci].opt()],
                            outs=[rs_out[ci].opt()],
                        )
                        rows = qb_per_chunk * P // n_cores
                        nc.sync.dma_start(
                            out_d[ci * rows : (ci + 1) * rows, :], rs_out[ci]
                        )

    nc.compile()
    return nc


def prepare_in_maps(x, W_qkv, W_out, cos, sin, mask, n_cores=N_CORES, hl=H // N_CORES):
    """Host-side sharding. Returns list of per-core input dicts."""
    t, d = x.shape
    x = np.asarray(x, dtype=BF16)
    W_qkv = np.asarray(W_qkv, dtype=BF16)
    W_out = np.asarray(W_out, dtype=BF16)
    cos = np.asarray(cos, dtype=np.float32)
    sin = np.asarray(sin, dtype=np.float32)
    m = np.asarray(mask, dtype=bool)

    xT = np.ascontiguousarray(x.T)
    cosT = np.ascontiguousarray(cos.T)
    sign = np.where(np.arange(DH) < DH // 2, -1.0, 1.0).astype(np.float32)
    ssinT = np.ascontiguousarray(sin.T * sign[:, None])
    rot = np.zeros((DH, DH), dtype=BF16)
    rot[(np.arange(DH) + DH // 2) % DH, np.arange(DH)] = 1

    mf = m.astype(np.float32)
    colmask = np.ascontiguousarray(
        np.broadcast_to(mf.astype(BF16)[None, :], (DH, t))
    )
    rqT = np.ascontiguousarray(
        np.where(mf, np.float32(0.0), np.float32(-1e9)).reshape(-1, DH).T
    )
    dvalT = np.ascontiguousarray((1.0 - mf).astype(np.float32).reshape(-1, DH).T)
    kch = 512
    p_idx = np.arange(DH)[:, None, None]
    v_idx = np.arange(4)[None, :, None]
    j_idx = np.arange(kch)[None, None, :]
    cmask = (v_idx * DH + p_idx >= j_idx).astype(BF16)

    n_heads = W_qkv.shape[1] // 3 // DH
    in_maps = []
    for c in range(n_cores):
        hs = [c * hl + i for i in range(hl)]
        cols = [W_qkv[:, (s * n_heads + h) * DH : (s * n_heads + h) * DH + DH]
                for s in range(3) for h in hs]
        wqkv_c = np.ascontiguousarray(np.concatenate(cols, axis=1))
        wout_c = np.ascontiguousarray(
            W_out[hs[0] * DH : (hs[-1] + 1) * DH, :]
        )
        in_maps.append(
            {
                "xT": xT,
                "wqkv": wqkv_c,
                "wout": wout_c,
                "cosT": cosT,
                "ssinT": ssinT,
                "rot": rot,
                "colmask": colmask,
                "rqT": rqT,
                "dvalT": dvalT,
                "cmask": cmask,
            }
        )
    return in_maps


_CACHED_NC = None


def assemble(results, t=T, d=D, n_cores=N_CORES, rs_chunks=4):
    """Reassemble per-core ReduceScatter slices into the full output."""
    rows_per_chunk = t // rs_chunks
    rows_per_core = rows_per_chunk // n_cores
    out = np.empty((t, d), dtype=BF16)
    for c in range(n_cores):
        oc = np.asarray(results[c]["out"])
        if oc.dtype != BF16:
            oc = oc.view(BF16)
        for ci in range(rs_chunks):
            lo = ci * rows_per_chunk + c * rows_per_core
            out[lo : lo + rows_per_core] = oc[
                ci * rows_per_core : (ci + 1) * rows_per_core
            ]
    return out


def kernel(x, W_qkv, W_out, cos, sin, mask):
    """Full inputs in, full output out. Shards across 8 NeuronCores."""
    global _CACHED_NC
    from concourse import bass_utils

    if _CACHED_NC is None:
        _CACHED_NC = build_nc()
    nc = _CACHED_NC

    in_maps = prepare_in_maps(x, W_qkv, W_out, cos, sin, mask)
    res = bass_utils.run_bass_kernel_spmd(
        nc, in_maps, core_ids=list(range(N_CORES))
    )
    return assemble(res.results)


# revision 25
# speedup vs baseline: 1.5488x; 1.1685x over previous
"""Distributed Trainium2 attention kernel (8 NeuronCores, head tensor-parallel).

Reference semantics (T=4096, D=2048, H=16, DH=128):
  qkv = bf16(x @ W_qkv); q,k,v per head; RoPE(split-half) on q,k;
  mask = ((m_q & m_k) | eye) & causal; softmax(q k^T / sqrt(DH) masked);
  out = bf16((probs @ v) @ W_out)

Sharding: head tensor-parallel. Core c owns heads (2c, 2c+1): W_qkv column
shard, W_out row shard, full x (replicated, passed pre-transposed).
Each core computes its heads' SDPA, its out-projection partial, then a
chunked ReduceScatter sums partials; core c emits output rows
[chunk*1024 + c*128 : +128) for each of the 4 chunks. Host reassembles.

Device-side layout choices:
  - x passed as xT [D, T] so the D contraction dim is the partition dim.
  - q,k computed weight-stationary -> born transposed [DH, T] (what the
    scores matmul needs); v transposed back to natural [T, DH] via PE.
  - RoPE: partition-rotate by 64 via a permutation-matrix matmul on PE,
    sign folded into a host-precomputed ssinT table; combine on DVE.
  - mask folded into a host-precomputed additive bias [T, T] bf16
    streamed from DRAM (0 where allowed, -1e9 where masked).
  - softmax without max-subtraction (scores are O(5) here); exp on ACT
    with accum_out producing the denominator in the same pass.
  - probs scaled by 1/denom, transposed 128x128 on PE; PV accumulates
    o^T [DH, T] directly, which is the lhsT layout out-proj wants.
"""

import os
import sys

import numpy as np

sys.path.insert(0, "/opt/trn_rl_repo")

import ml_dtypes

BF16 = ml_dtypes.bfloat16

# problem constants (hardcoded per harness contract)
T, D, H, DH = 4096, 2048, 16, 128
N_CORES = 8
ROPE_BASE = 10000.0


def build_nc(
    t=T,
    d=D,
    n_cores=N_CORES,
    hl=H // N_CORES,  # heads per core
    kch=512,  # scores k-chunk (free dim of scores matmul)
    tch=512,  # qkv t-chunk
    rs_chunks=4,  # reduce-scatter chunks
):
    import concourse.bass as bass
    import concourse.mybir as mybir
    import concourse.tile as tile
    from concourse import bacc
    from concourse.masks import make_identity

    f32 = mybir.dt.float32
    bf16 = mybir.dt.bfloat16

    P = 128
    kd = d // P  # contraction chunks for qkv
    qb_n = t // P  # q-blocks of 128 rows
    nt = t // tch  # t-chunks in qkv phase
    jl = hl * P  # local out-proj contraction width
    qb_per_chunk = qb_n // rs_chunks
    t_out = t // n_cores  # output rows per core
    scale = 1.0 / np.sqrt(DH)

    nc = bacc.Bacc(
        "TRN2", target_bir_lowering=False, debug=False, num_devices=n_cores
    )

    xT = nc.dram_tensor("xT", [d, t], bf16, kind="ExternalInput").ap()
    wqkv = nc.dram_tensor("wqkv", [d, 3 * jl], bf16, kind="ExternalInput").ap()
    wout_d = nc.dram_tensor("wout", [jl, d], bf16, kind="ExternalInput").ap()
    cosT_d = nc.dram_tensor("cosT", [P, t], f32, kind="ExternalInput").ap()
    ssinT_d = nc.dram_tensor("ssinT", [P, t], f32, kind="ExternalInput").ap()
    rot_d = nc.dram_tensor("rot", [P, P], bf16, kind="ExternalInput").ap()
    # colmask: mask[k] as 0/1 bf16, pre-broadcast to all 128 partitions
    colmask_d = nc.dram_tensor("colmask", [P, t], bf16, kind="ExternalInput").ap()
    # rqT[p, qb] = 0 if mask[qb*128+p] else -1e9 (folded into exp bias)
    rqT_d = nc.dram_tensor("rqT", [P, qb_n], f32, kind="ExternalInput").ap()
    # dvalB[p, q] = 1 - mask[q], broadcast to all partitions
    dvalB_d = nc.dram_tensor("dvalB", [P, t], bf16, kind="ExternalInput").ap()
    # cmask128[p, j] = 1 if j >= p else 0 (within-block causal, T-orientation)
    cmask128_d = nc.dram_tensor("cmask128", [P, P], bf16, kind="ExternalInput").ap()
    out_d = nc.dram_tensor("out", [t_out, d], bf16, kind="ExternalOutput").ap()

    with tile.TileContext(nc) as tc:
        with tc.tile_pool(name="persist", bufs=1) as persist:
            # persistent SBUF tensors
            ident = persist.tile([P, P], bf16, name="ident")
            make_identity(nc, ident)
            rot_sb = persist.tile([P, P], bf16, name="rot_sb")
            nc.sync.dma_start(rot_sb, rot_d)
            wq_sb = persist.tile([P, kd, 3 * hl, P], bf16, name="wq_sb")
            nc.sync.dma_start(
                wq_sb, wqkv.rearrange("(kd p) (c j) -> p kd c j", p=P, j=P)
            )
            wout_sb = persist.tile([P, hl, d], bf16, name="wout_sb")
            nc.sync.dma_start(wout_sb, wout_d.rearrange("(h p) x -> p h x", p=P))

            # per-head persistent activations
            qT = [persist.tile([P, t], bf16, name=f"qT{h}") for h in range(hl)]
            kT = [persist.tile([P, t], bf16, name=f"kT{h}") for h in range(hl)]
            vT = [persist.tile([P, t], bf16, name=f"vT{h}") for h in range(hl)]
            v_nat = [
                persist.tile([P, qb_n, P], bf16, name=f"vnat{h}") for h in range(hl)
            ]
            oT = [persist.tile([P, t], bf16, name=f"oT{h}") for h in range(hl)]

            # ---------------- phase 1: qkv + rope + v transpose ----------
            with (
                tc.tile_pool(name="ph1", bufs=2) as ph1,
                tc.tile_pool(name="cs", bufs=1) as cspool,
                tc.tile_pool(name="ps_qkv", bufs=1, space="PSUM") as ps_qkv,
                tc.tile_pool(name="ps_aux", bufs=2, space="PSUM") as ps_aux,
            ):
                cosT_sb = cspool.tile([P, t], f32, name="cosT_sb")
                nc.sync.dma_start(cosT_sb, cosT_d)
                ssinT_sb = cspool.tile([P, t], f32, name="ssinT_sb")
                nc.sync.dma_start(ssinT_sb, ssinT_d)

                for tc_i in range(nt):
                    tsl = slice(tc_i * tch, (tc_i + 1) * tch)
                    xt = ph1.tile([P, kd, tch], bf16, tag="xt")
                    nc.sync.dma_start(
                        xt, xT.rearrange("(kd p) x -> p kd x", p=P)[:, :, tsl]
                    )
                    for c in range(3 * hl):  # q0,q1,k0,k1,v0,v1
                        ps = ps_qkv.tile([P, tch], mybir.dt.float32, tag=f"ps{c}")
                        for k in range(kd):
                            nc.tensor.matmul(
                                ps,
                                lhsT=wq_sb[:, k, c],
                                rhs=xt[:, k],
                                start=(k == 0),
                                stop=(k == kd - 1),
                            )
                        if c < 2 * hl:  # q or k: cast, rotate, rope-combine
                            dst = qT[c] if c < hl else kT[c - hl]
                            qbf = ph1.tile([P, tch], bf16, tag="qbf")
                            nc.scalar.copy(qbf, ps)
                            shift = ps_aux.tile(
                                [P, tch], mybir.dt.float32, tag="aux"
                            )
                            nc.tensor.matmul(
                                shift, lhsT=rot_sb, rhs=qbf, start=True, stop=True
                            )
                            t1 = ph1.tile([P, tch], f32, tag="t1")
                            nc.vector.tensor_tensor(
                                t1, qbf, cosT_sb[:, tsl], mybir.AluOpType.mult
                            )
                            t2 = ph1.tile([P, tch], f32, tag="t2")
                            nc.vector.tensor_tensor(
                                t2, shift, ssinT_sb[:, tsl], mybir.AluOpType.mult
                            )
                            nc.vector.tensor_tensor(
                                dst[:, tsl], t1, t2, mybir.AluOpType.add
                            )
                        else:  # v: just cast
                            nc.scalar.copy(vT[c - 2 * hl][:, tsl], ps)

                # v: [DH, T] -> natural [T-block, DH] tiles
                for h in range(hl):
                    for b in range(qb_n):
                        pst = ps_aux.tile([P, P], bf16, tag="aux")
                        nc.tensor.transpose(
                            pst, vT[h][:, b * P : (b + 1) * P], ident
                        )
                        nc.scalar.copy(v_nat[h][:, b], pst)

            # ---------------- phase 2: SDPA + out-proj + RS --------------
            # Transposed-scores formulation: scoresT[k, q] tiles per 128-k
            # block over a 512-query "quad"; exp evacuates psum straight to
            # the PV rhs; denominator via a ones-column matmul; softmax
            # normalization + masked-row fixup fused into the single oT
            # evacuation (per-query scalars partition-broadcast on gpsimd).
            qw = 512  # queries per quad
            with (
                tc.tile_pool(name="ph2", bufs=3) as ph2,
                tc.tile_pool(name="pt", bufs=3) as ptpool,
                tc.tile_pool(name="msk", bufs=1) as mskpool,
                tc.tile_pool(name="dram", bufs=1, space="DRAM") as dram,
                tc.tile_pool(name="ps_s", bufs=3, space="PSUM") as ps_s,
                tc.tile_pool(name="ps_o", bufs=2, space="PSUM") as ps_o,
                tc.tile_pool(name="ps_d", bufs=2, space="PSUM") as ps_d,
                tc.tile_pool(name="ps_out", bufs=1, space="PSUM") as ps_out,
            ):
                colmask_sb = mskpool.tile([P, t], bf16, name="colmask_sb")
                nc.sync.dma_start(colmask_sb, colmask_d)
                rqT_sb = mskpool.tile([P, qb_n], f32, name="rqT_sb")
                nc.sync.dma_start(rqT_sb, rqT_d)
                dvalB_sb = mskpool.tile([P, t], bf16, name="dvalB_sb")
                nc.sync.dma_start(dvalB_sb, dvalB_d)
                cm128_sb = mskpool.tile([P, P], bf16, name="cm128_sb")
                nc.sync.dma_start(cm128_sb, cmask128_d)
                ones_sb = mskpool.tile([P, 1], bf16, name="ones_sb")
                nc.vector.memset(ones_sb, 1.0)
                rs_in = [
                    dram.tile([qb_per_chunk * P * n_cores // n_cores, d], bf16,
                              name=f"rs_in{ci}")
                    for ci in range(rs_chunks)
                ]
                rs_out = [
                    dram.tile([qb_per_chunk * P // n_cores, d], bf16,
                              name=f"rs_out{ci}")
                    for ci in range(rs_chunks)
                ]

                n_quads = t // qw
                qb_per_quad = qw // P  # 4
                for g in range(n_quads):
                    gsl = slice(g * qw, (g + 1) * qw)
                    nsk = (g + 1) * qb_per_quad  # causal k-blocks for quad
                    for h in range(hl):
                        pso = ps_o.tile([P, qw], f32, tag="oT")
                        psd = ps_d.tile([1, qw], f32, tag="den")
                        for sk in range(nsk):
                            br = sk - g * qb_per_quad  # >=0 in diag region
                            lo = br * P if br >= 0 else 0
                            psT = ps_s.tile([P, qw], f32, tag="scT")
                            nc.tensor.matmul(
                                psT[:, lo:],
                                lhsT=kT[h][:, sk * P : (sk + 1) * P],
                                rhs=qT[h][:, g * qw + lo : (g + 1) * qw],
                                start=True,
                                stop=True,
                            )
                            pT = ptpool.tile([P, qw], bf16, tag="pT")
                            # exp; per-k padding mask folded into the bias
                            nc.scalar.activation(
                                pT[:, lo:],
                                psT[:, lo:],
                                mybir.ActivationFunctionType.Exp,
                                scale=float(scale),
                                bias=rqT_sb[:, sk : sk + 1],
                            )
                            if br >= 0:
                                # within-block causal on the partial 128 cols
                                nc.vector.tensor_tensor(
                                    pT[:, lo : lo + P],
                                    pT[:, lo : lo + P],
                                    cm128_sb,
                                    mybir.AluOpType.mult,
                                )
                            nc.tensor.matmul(
                                pso[:, lo:],
                                lhsT=v_nat[h][:, sk],
                                rhs=pT[:, lo:],
                                start=(sk == 0),
                                stop=(sk == nsk - 1),
                            )
                            nc.tensor.matmul(
                                psd[:, lo:],
                                lhsT=ones_sb,
                                rhs=pT[:, lo:],
                                start=(sk == 0),
                                stop=(sk == nsk - 1),
                            )
                        # denominators -> inv, zeroed for masked queries
                        dsum = ph2.tile([1, qw], f32, tag="dsum")
                        nc.vector.tensor_copy(dsum, psd)
                        inv = ph2.tile([1, qw], f32, tag="inv")
                        nc.vector.reciprocal(inv, dsum)
                        inv2 = ph2.tile([1, qw], f32, tag="inv2")
                        nc.vector.tensor_tensor(
                            inv2, inv, colmask_sb[0:1, gsl], mybir.AluOpType.mult
                        )
                        invB = ph2.tile([P, qw], f32, tag="invB")
                        nc.gpsimd.partition_broadcast(invB, inv2)
                        # oT = pso * inv[q] + v^T * (1-m[q])  (masked queries
                        # attend only themselves -> o = v)
                        nc.vector.tensor_tensor(
                            oT[h][:, gsl], pso, invB, mybir.AluOpType.mult
                        )
                        vblend = ph2.tile([P, qw], bf16, tag="vblend")
                        nc.gpsimd.tensor_tensor(
                            vblend, vT[h][:, gsl], dvalB_sb[:, gsl],
                            mybir.AluOpType.mult,
                        )
                        nc.vector.tensor_tensor(
                            oT[h][:, gsl], oT[h][:, gsl], vblend,
                            mybir.AluOpType.add,
                        )

                    # out-projection + RS for the quad's 4 q-blocks
                    for qq in range(qb_per_quad):
                        qb = g * qb_per_quad + qq
                        qsl = slice(qb * P, (qb + 1) * P)
                        partial = ph2.tile([P, d], bf16, tag="partial")
                        for ntile in range(d // 512):
                            nsl = slice(ntile * 512, (ntile + 1) * 512)
                            pso2 = ps_out.tile([P, 512], f32, tag="outp")
                            for h in range(hl):
                                nc.tensor.matmul(
                                    pso2,
                                    lhsT=oT[h][:, qsl],
                                    rhs=wout_sb[:, h, nsl],
                                    start=(h == 0),
                                    stop=(h == hl - 1),
                                )
                            nc.vector.tensor_copy(partial[:, nsl], pso2)
                        ci = qb // qb_per_chunk
                        ri = qb % qb_per_chunk
                        nc.sync.dma_start(
                            rs_in[ci][ri * P : (ri + 1) * P, :], partial
                        )
                        if ri == qb_per_chunk - 1:
                            nc.gpsimd.collective_compute(
                                "ReduceScatter",
                                mybir.AluOpType.add,
                                replica_groups=[list(range(n_cores))],
                                ins=[rs_in[ci].opt()],
                                outs=[rs_out[ci].opt()],
                            )
                            rows = qb_per_chunk * P // n_cores
                            nc.sync.dma_start(
                                out_d[ci * rows : (ci + 1) * rows, :],
                                rs_out[ci],
                            )

    nc.compile()
    return nc


def prepare_in_maps(x, W_qkv, W_out, cos, sin, mask, n_cores=N_CORES, hl=H // N_CORES):
    """Host-side sharding. Returns list of per-core input dicts."""
    t, d = x.shape
    x = np.asarray(x, dtype=BF16)
    W_qkv = np.asarray(W_qkv, dtype=BF16)
    W_out = np.asarray(W_out, dtype=BF16)
    cos = np.asarray(cos, dtype=np.float32)
    sin = np.asarray(sin, dtype=np.float32)
    m = np.asarray(mask, dtype=bool)

    xT = np.ascontiguousarray(x.T)
    cosT = np.ascontiguousarray(cos.T)
    sign = np.where(np.arange(DH) < DH // 2, -1.0, 1.0).astype(np.float32)
    ssinT = np.ascontiguousarray(sin.T * sign[:, None])
    rot = np.zeros((DH, DH), dtype=BF16)
    rot[(np.arange(DH) + DH // 2) % DH, np.arange(DH)] = 1

    mf = m.astype(np.float32)
    colmask = np.ascontiguousarray(
        np.broadcast_to(mf.astype(BF16)[None, :], (DH, t))
    )
    rqT = np.ascontiguousarray(
        np.where(mf, np.float32(0.0), np.float32(-1e9)).reshape(-1, DH).T
    )
    dvalB = np.ascontiguousarray(
        np.broadcast_to((1.0 - mf).astype(BF16)[None, :], (DH, t))
    )
    cmask128 = (np.arange(DH)[None, :] >= np.arange(DH)[:, None]).astype(BF16)

    n_heads = W_qkv.shape[1] // 3 // DH
    in_maps = []
    for c in range(n_cores):
        hs = [c * hl + i for i in range(hl)]
        cols = [W_qkv[:, (s * n_heads + h) * DH : (s * n_heads + h) * DH + DH]
                for s in range(3) for h in hs]
        wqkv_c = np.ascontiguousarray(np.concatenate(cols, axis=1))
        wout_c = np.ascontiguousarray(
            W_out[hs[0] * DH : (hs[-1] + 1) * DH, :]
        )
        in_maps.append(
            {
                "xT": xT,
                "wqkv": wqkv_c,
                "wout": wout_c,
                "cosT": cosT,
                "ssinT": ssinT,
                "rot": rot,
                "colmask": colmask,
                "rqT": rqT,
                "dvalB": dvalB,
                "cmask128": cmask128,
            }
        )
    return in_maps


_CACHED_NC = None


def assemble(results, t=T, d=D, n_cores=N_CORES, rs_chunks=4):
    """Reassemble per-core ReduceScatter slices into the full output."""
    rows_per_chunk = t // rs_chunks
    rows_per_core = rows_per_chunk // n_cores
    out = np.empty((t, d), dtype=BF16)
    for c in range(n_cores):
        oc = np.asarray(results[c]["out"])
        if oc.dtype != BF16:
            oc = oc.view(BF16)
        for ci in range(rs_chunks):
            lo = ci * rows_per_chunk + c * rows_per_core
            out[lo : lo + rows_per_core] = oc[
                ci * rows_per_core : (ci + 1) * rows_per_core
            ]
    return out


def kernel(x, W_qkv, W_out, cos, sin, mask):
    """Full inputs in, full output out. Shards across 8 NeuronCores."""
    global _CACHED_NC
    from concourse import bass_utils

    if _CACHED_NC is None:
        _CACHED_NC = build_nc()
    nc = _CACHED_NC

    in_maps = prepare_in_maps(x, W_qkv, W_out, cos, sin, mask)
    res = bass_utils.run_bass_kernel_spmd(
        nc, in_maps, core_ids=list(range(N_CORES))
    )
    return assemble(res.results)


# revision 32
# speedup vs baseline: 1.7375x; 1.1218x over previous
"""Distributed Trainium2 attention kernel (8 NeuronCores, head tensor-parallel).

Reference semantics (T=4096, D=2048, H=16, DH=128):
  qkv = bf16(x @ W_qkv); q,k,v per head; RoPE(split-half) on q,k;
  mask = ((m_q & m_k) | eye) & causal; softmax(q k^T / sqrt(DH) masked);
  out = bf16((probs @ v) @ W_out)

Sharding: head tensor-parallel. Core c owns heads (2c, 2c+1): W_qkv column
shard, W_out row shard, full x (replicated, passed pre-transposed).
Each core computes its heads' SDPA, its out-projection partial, then a
chunked ReduceScatter sums partials; core c emits output rows
[chunk*1024 + c*128 : +128) for each of the 4 chunks. Host reassembles.

Device-side layout choices:
  - x passed as xT [D, T] so the D contraction dim is the partition dim.
  - q,k computed weight-stationary -> born transposed [DH, T]; v
    transposed back to natural [T, DH] via PE (PV lhsT layout).
  - RoPE: partition-rotate by 64 via a permutation-matrix matmul on PE,
    sign folded into a host-precomputed ssinT table; combine on DVE.
  - SDPA in transposed-scores form: scoresT[k, q] tiles over 512-query
    quads; exp (no max-subtraction; scores are O(5) here) evacuates the
    scores psum straight into the PV rhs -- no probs transposes.
  - key padding mask folded into the exp bias (per-k = per-partition);
    within-block causal via one precomputed 0/1 [128,128] multiply.
  - softmax denominators via a ones-column matmul, transposed to
    q-partition layout with 4 tiny K=1 matmuls; normalization deferred
    to the out-projection epilogue (per-partition scalars there), with
    a mid-accumulation ratio scale to handle the two heads' different
    denominators in one psum group.
  - masked queries (attend only self) fixed by blending v^T via colmask
    zeroing + (1-m) add; all blend scalars stay per-partition.
"""

import os
import sys

import numpy as np

sys.path.insert(0, "/opt/trn_rl_repo")

import ml_dtypes

BF16 = ml_dtypes.bfloat16

# problem constants (hardcoded per harness contract)
T, D, H, DH = 4096, 2048, 16, 128
N_CORES = 8
ROPE_BASE = 10000.0


def build_nc(
    t=T,
    d=D,
    n_cores=N_CORES,
    hl=H // N_CORES,  # heads per core
    kch=512,  # scores k-chunk (free dim of scores matmul)
    tch=512,  # qkv t-chunk
    rs_chunks=8,  # reduce-scatter chunks
):
    import concourse.bass as bass
    import concourse.mybir as mybir
    import concourse.tile as tile
    from concourse import bacc
    from concourse.masks import make_identity

    f32 = mybir.dt.float32
    bf16 = mybir.dt.bfloat16

    P = 128
    kd = d // P  # contraction chunks for qkv
    qb_n = t // P  # q-blocks of 128 rows
    nt = t // tch  # t-chunks in qkv phase
    jl = hl * P  # local out-proj contraction width
    qb_per_chunk = qb_n // rs_chunks
    t_out = t // n_cores  # output rows per core
    scale = 1.0 / np.sqrt(DH)

    nc = bacc.Bacc(
        "TRN2", target_bir_lowering=False, debug=False, num_devices=n_cores
    )

    xT = nc.dram_tensor("xT", [d, t], bf16, kind="ExternalInput").ap()
    wqkv = nc.dram_tensor("wqkv", [d, 3 * jl], bf16, kind="ExternalInput").ap()
    wout_d = nc.dram_tensor("wout", [jl, d], bf16, kind="ExternalInput").ap()
    cosT_d = nc.dram_tensor("cosT", [P, t], f32, kind="ExternalInput").ap()
    ssinT_d = nc.dram_tensor("ssinT", [P, t], f32, kind="ExternalInput").ap()
    rot_d = nc.dram_tensor("rot", [P, P], bf16, kind="ExternalInput").ap()
    # colmask: mask[k] as 0/1 bf16, pre-broadcast to all 128 partitions
    colmask_d = nc.dram_tensor("colmask", [P, t], bf16, kind="ExternalInput").ap()
    # rqT[p, qb] = 0 if mask[qb*128+p] else -1e9 (folded into exp bias)
    rqT_d = nc.dram_tensor("rqT", [P, qb_n], f32, kind="ExternalInput").ap()
    # dvalB[p, q] = 1 - mask[q], broadcast to all partitions
    dvalB_d = nc.dram_tensor("dvalB", [P, t], bf16, kind="ExternalInput").ap()
    # dvalT[p, qb] = 1 - mask[qb*128+p] (q-partition layout)
    dvalT_d = nc.dram_tensor("dvalT", [P, qb_n], f32, kind="ExternalInput").ap()
    # mqT[p, qb] = mask[qb*128+p] (q-partition layout)
    mqT_d = nc.dram_tensor("mqT", [P, qb_n], f32, kind="ExternalInput").ap()
    # cmask128[p, j] = 1 if j >= p else 0 (within-block causal, T-orientation)
    cmask128_d = nc.dram_tensor("cmask128", [P, P], bf16, kind="ExternalInput").ap()
    out_d = nc.dram_tensor("out", [t_out, d], bf16, kind="ExternalOutput").ap()

    with tile.TileContext(nc) as tc:
        with tc.tile_pool(name="persist", bufs=1) as persist:
            # persistent SBUF tensors
            ident = persist.tile([P, P], bf16, name="ident")
            make_identity(nc, ident)
            rot_sb = persist.tile([P, P], bf16, name="rot_sb")
            nc.sync.dma_start(rot_sb, rot_d)
            wq_sb = persist.tile([P, kd, 3 * hl, P], bf16, name="wq_sb")
            nc.sync.dma_start(
                wq_sb, wqkv.rearrange("(kd p) (c j) -> p kd c j", p=P, j=P)
            )
            wout_sb = persist.tile([P, hl, d], bf16, name="wout_sb")
            nc.sync.dma_start(wout_sb, wout_d.rearrange("(h p) x -> p h x", p=P))

            # per-head persistent activations
            qT = [persist.tile([P, t], bf16, name=f"qT{h}") for h in range(hl)]
            kT = [persist.tile([P, t], bf16, name=f"kT{h}") for h in range(hl)]
            vT = [persist.tile([P, t], bf16, name=f"vT{h}") for h in range(hl)]
            v_nat = [
                persist.tile([P, qb_n, P], bf16, name=f"vnat{h}") for h in range(hl)
            ]
            oT = [persist.tile([P, t], bf16, name=f"oT{h}") for h in range(hl)]

            # ---------------- phase 1: qkv + rope + v transpose ----------
            with (
                tc.tile_pool(name="ph1", bufs=2) as ph1,
                tc.tile_pool(name="cs", bufs=1) as cspool,
                tc.tile_pool(name="ps_qkv", bufs=1, space="PSUM") as ps_qkv,
                tc.tile_pool(name="ps_aux", bufs=2, space="PSUM") as ps_aux,
            ):
                cosT_sb = cspool.tile([P, t], f32, name="cosT_sb")
                nc.sync.dma_start(cosT_sb, cosT_d)
                ssinT_sb = cspool.tile([P, t], f32, name="ssinT_sb")
                nc.sync.dma_start(ssinT_sb, ssinT_d)

                for tc_i in range(nt):
                    tsl = slice(tc_i * tch, (tc_i + 1) * tch)
                    xt = ph1.tile([P, kd, tch], bf16, tag="xt")
                    nc.sync.dma_start(
                        xt, xT.rearrange("(kd p) x -> p kd x", p=P)[:, :, tsl]
                    )
                    for c in range(3 * hl):  # q0,q1,k0,k1,v0,v1
                        ps = ps_qkv.tile([P, tch], mybir.dt.float32, tag=f"ps{c}")
                        for k in range(kd):
                            nc.tensor.matmul(
                                ps,
                                lhsT=wq_sb[:, k, c],
                                rhs=xt[:, k],
                                start=(k == 0),
                                stop=(k == kd - 1),
                            )
                        if c < 2 * hl:  # q or k: cast, rotate, rope-combine
                            dst = qT[c] if c < hl else kT[c - hl]
                            qbf = ph1.tile([P, tch], bf16, tag="qbf")
                            nc.scalar.copy(qbf, ps)
                            shift = ps_aux.tile(
                                [P, tch], mybir.dt.float32, tag="aux"
                            )
                            nc.tensor.matmul(
                                shift, lhsT=rot_sb, rhs=qbf, start=True, stop=True
                            )
                            t1 = ph1.tile([P, tch], f32, tag="t1")
                            nc.vector.tensor_tensor(
                                t1, qbf, cosT_sb[:, tsl], mybir.AluOpType.mult
                            )
                            t2 = ph1.tile([P, tch], f32, tag="t2")
                            nc.vector.tensor_tensor(
                                t2, shift, ssinT_sb[:, tsl], mybir.AluOpType.mult
                            )
                            nc.vector.tensor_tensor(
                                dst[:, tsl], t1, t2, mybir.AluOpType.add
                            )
                        else:  # v: just cast
                            nc.scalar.copy(vT[c - 2 * hl][:, tsl], ps)

                # v: [DH, T] -> natural [T-block, DH] tiles
                for h in range(hl):
                    for b in range(qb_n):
                        pst = ps_aux.tile([P, P], bf16, tag="aux")
                        nc.tensor.transpose(
                            pst, vT[h][:, b * P : (b + 1) * P], ident
                        )
                        nc.scalar.copy(v_nat[h][:, b], pst)

            # ---------------- phase 2: SDPA + out-proj + RS --------------
            # Transposed-scores formulation: scoresT[k, q] tiles per 128-k
            # block over a 512-query "quad"; exp evacuates psum straight to
            # the PV rhs; denominator via a ones-column matmul; softmax
            # normalization + masked-row fixup fused into the single oT
            # evacuation (per-query scalars partition-broadcast on gpsimd).
            qw = 512  # queries per quad
            with (
                tc.tile_pool(name="ph2", bufs=3) as ph2,
                tc.tile_pool(name="pt", bufs=3) as ptpool,
                tc.tile_pool(name="msk", bufs=1) as mskpool,
                tc.tile_pool(name="dram", bufs=1, space="DRAM") as dram,
                tc.tile_pool(name="ps_s", bufs=2, space="PSUM") as ps_s,
                tc.tile_pool(name="ps_o", bufs=1, space="PSUM") as ps_o,
                tc.tile_pool(name="ps_d", bufs=2, space="PSUM") as ps_d,
                tc.tile_pool(name="ps_dt", bufs=1, space="PSUM") as ps_dt,
                tc.tile_pool(name="ps_out", bufs=2, space="PSUM") as ps_out,
            ):
                colmask_sb = mskpool.tile([P, t], bf16, name="colmask_sb")
                nc.sync.dma_start(colmask_sb, colmask_d)
                rqT_sb = mskpool.tile([P, qb_n], f32, name="rqT_sb")
                nc.sync.dma_start(rqT_sb, rqT_d)
                dvalB_sb = mskpool.tile([P, t], bf16, name="dvalB_sb")
                nc.sync.dma_start(dvalB_sb, dvalB_d)
                cm128_sb = mskpool.tile([P, P], bf16, name="cm128_sb")
                nc.sync.dma_start(cm128_sb, cmask128_d)
                ones_sb = mskpool.tile([P, 1], bf16, name="ones_sb")
                nc.vector.memset(ones_sb, 1.0)
                dvalT_sb = mskpool.tile([P, qb_n], f32, name="dvalT_sb")
                nc.sync.dma_start(dvalT_sb, dvalT_d)
                mqT_sb = mskpool.tile([P, qb_n], f32, name="mqT_sb")
                nc.sync.dma_start(mqT_sb, mqT_d)
                rs_in = [
                    dram.tile([qb_per_chunk * P * n_cores // n_cores, d], bf16,
                              name=f"rs_in{ci}")
                    for ci in range(rs_chunks)
                ]
                rs_out = [
                    dram.tile([qb_per_chunk * P // n_cores, d], bf16,
                              name=f"rs_out{ci}")
                    for ci in range(rs_chunks)
                ]

                n_quads = t // qw
                qb_per_quad = qw // P  # 4
                for g in range(n_quads):
                    gsl = slice(g * qw, (g + 1) * qw)
                    nsk = (g + 1) * qb_per_quad  # causal k-blocks for quad
                    for h in range(hl):
                        pso = ps_o.tile([P, qw], f32, tag="oT")
                        psd = ps_d.tile([1, qw], f32, tag="den")
                        for sk in range(nsk):
                            br = sk - g * qb_per_quad  # >=0 in diag region
                            lo = br * P if br >= 0 else 0
                            psT = ps_s.tile([P, qw], f32, tag="scT")
                            nc.tensor.matmul(
                                psT[:, lo:],
                                lhsT=kT[h][:, sk * P : (sk + 1) * P],
                                rhs=qT[h][:, g * qw + lo : (g + 1) * qw],
                                start=True,
                                stop=True,
                            )
                            pT = ptpool.tile([P, qw], bf16, tag="pT")
                            # exp; per-k padding mask folded into the bias
                            nc.scalar.activation(
                                pT[:, lo:],
                                psT[:, lo:],
                                mybir.ActivationFunctionType.Exp,
                                scale=float(scale),
                                bias=rqT_sb[:, sk : sk + 1],
                            )
                            if br >= 0:
                                # within-block causal on the partial 128 cols
                                nc.vector.tensor_tensor(
                                    pT[:, lo : lo + P],
                                    pT[:, lo : lo + P],
                                    cm128_sb,
                                    mybir.AluOpType.mult,
                                )
                            nc.tensor.matmul(
                                pso[:, lo:],
                                lhsT=v_nat[h][:, sk],
                                rhs=pT[:, lo:],
                                start=(sk == 0),
                                stop=(sk == nsk - 1),
                            )
                            nc.tensor.matmul(
                                psd[:, lo:],
                                lhsT=ones_sb,
                                rhs=pT[:, lo:],
                                start=(sk == 0),
                                stop=(sk == nsk - 1),
                            )
                        # oT kept UNNORMALIZED (bf16 is scale-free); masked-q
                        # garbage zeroed via colmask; masked queries attend
                        # only themselves -> blend in v^T * (1-m[q])
                        nc.vector.tensor_tensor(
                            oT[h][:, gsl], pso, colmask_sb[:, gsl],
                            mybir.AluOpType.mult,
                        )
                        vblend = ph2.tile([P, qw], bf16, tag="vblend")
                        nc.gpsimd.tensor_tensor(
                            vblend, vT[h][:, gsl], dvalB_sb[:, gsl],
                            mybir.AluOpType.mult,
                        )
                        nc.vector.tensor_tensor(
                            oT[h][:, gsl], oT[h][:, gsl], vblend,
                            mybir.AluOpType.add,
                        )
                        # denominators -> q-partition layout: transpose the
                        # [1, 512] row into [128, 4] via 4 tiny K=1 matmuls,
                        # so the reciprocal runs on all 128 lanes
                        dsum_bf = ph2.tile([1, qw], bf16, tag=f"dsum{h}")
                        nc.vector.tensor_copy(dsum_bf, psd)
                        denT = ps_dt.tile([P, qb_per_quad], f32, tag="denT")
                        for j in range(qb_per_quad):
                            nc.tensor.matmul(
                                denT[:, j : j + 1],
                                lhsT=dsum_bf[0:1, j * P : (j + 1) * P],
                                rhs=ones_sb[0:1, 0:1],
                                start=True,
                                stop=True,
                            )
                        gq = slice(g * qb_per_quad, (g + 1) * qb_per_quad)
                        rec = ph2.tile([P, qb_per_quad], f32, tag=f"rec{h}")
                        nc.vector.reciprocal(rec, denT)
                        # inv3 = m[q]/denom + (1-m[q])  (masked rows pass the
                        # blended v through unscaled)
                        inv3 = ph2.tile([P, qb_per_quad], f32, tag=f"inv3{h}")
                        nc.vector.tensor_tensor(
                            inv3, rec, mqT_sb[:, gq], mybir.AluOpType.mult
                        )
                        nc.vector.tensor_tensor(
                            inv3, inv3, dvalT_sb[:, gq], mybir.AluOpType.add
                        )
                        if h == hl - 1 and hl > 1:
                            # jnv = m*denom + (1-m); r = inv3_0 * jnv_1 so a
                            # mid-accumulation psum scale by r followed by a
                            # final scale by inv3_1 yields per-head softmax
                            # normalization inside one accumulation group
                            jnv = ph2.tile([P, qb_per_quad], f32, tag="jnv")
                            nc.vector.tensor_tensor(
                                jnv, denT, mqT_sb[:, gq], mybir.AluOpType.mult
                            )
                            nc.vector.tensor_tensor(
                                jnv, jnv, dvalT_sb[:, gq], mybir.AluOpType.add
                            )
                            rsc = ph2.tile([P, qb_per_quad], f32, tag="rsc")
                            nc.vector.tensor_tensor(
                                rsc, inv3_prev, jnv, mybir.AluOpType.mult
                            )
                        inv3_prev = inv3

                    # out-projection + RS for the quad's 4 q-blocks
                    for qq in range(qb_per_quad):
                        qb = g * qb_per_quad + qq
                        qsl = slice(qb * P, (qb + 1) * P)
                        partial = ph2.tile([P, d], bf16, tag="partial")
                        for ntile in range(d // 512):
                            nsl = slice(ntile * 512, (ntile + 1) * 512)
                            pso2 = ps_out.tile([P, 512], f32, tag="outp")
                            for h in range(hl):
                                nc.tensor.matmul(
                                    pso2,
                                    lhsT=oT[h][:, qsl],
                                    rhs=wout_sb[:, h, nsl],
                                    start=(h == 0),
                                    stop=True,
                                    skip_group_check=(h > 0),
                                )
                                if h < hl - 1:
                                    nc.vector.tensor_scalar_mul(
                                        pso2, pso2, rsc[:, qq : qq + 1]
                                    )
                            nc.vector.tensor_scalar_mul(
                                partial[:, nsl], pso2, inv3[:, qq : qq + 1]
                            )
                        ci = qb // qb_per_chunk
                        ri = qb % qb_per_chunk
                        nc.sync.dma_start(
                            rs_in[ci][ri * P : (ri + 1) * P, :], partial
                        )
                        if ri == qb_per_chunk - 1:
                            nc.gpsimd.collective_compute(
                                "ReduceScatter",
                                mybir.AluOpType.add,
                                replica_groups=[list(range(n_cores))],
                                ins=[rs_in[ci].opt()],
                                outs=[rs_out[ci].opt()],
                            )
                            rows = qb_per_chunk * P // n_cores
                            nc.sync.dma_start(
                                out_d[ci * rows : (ci + 1) * rows, :],
                                rs_out[ci],
                            )

    nc.compile()
    return nc


def prepare_in_maps(x, W_qkv, W_out, cos, sin, mask, n_cores=N_CORES, hl=H // N_CORES):
    """Host-side sharding. Returns list of per-core input dicts."""
    t, d = x.shape
    x = np.asarray(x, dtype=BF16)
    W_qkv = np.asarray(W_qkv, dtype=BF16)
    W_out = np.asarray(W_out, dtype=BF16)
    cos = np.asarray(cos, dtype=np.float32)
    sin = np.asarray(sin, dtype=np.float32)
    m = np.asarray(mask, dtype=bool)

    xT = np.ascontiguousarray(x.T)
    cosT = np.ascontiguousarray(cos.T)
    sign = np.where(np.arange(DH) < DH // 2, -1.0, 1.0).astype(np.float32)
    ssinT = np.ascontiguousarray(sin.T * sign[:, None])
    rot = np.zeros((DH, DH), dtype=BF16)
    rot[(np.arange(DH) + DH // 2) % DH, np.arange(DH)] = 1

    mf = m.astype(np.float32)
    colmask = np.ascontiguousarray(
        np.broadcast_to(mf.astype(BF16)[None, :], (DH, t))
    )
    rqT = np.ascontiguousarray(
        np.where(mf, np.float32(0.0), np.float32(-1e9)).reshape(-1, DH).T
    )
    dvalB = np.ascontiguousarray(
        np.broadcast_to((1.0 - mf).astype(BF16)[None, :], (DH, t))
    )
    dvalT = np.ascontiguousarray((1.0 - mf).astype(np.float32).reshape(-1, DH).T)
    mqT = np.ascontiguousarray(mf.astype(np.float32).reshape(-1, DH).T)
    cmask128 = (np.arange(DH)[None, :] >= np.arange(DH)[:, None]).astype(BF16)

    n_heads = W_qkv.shape[1] // 3 // DH
    in_maps = []
    for c in range(n_cores):
        hs = [c * hl + i for i in range(hl)]
        cols = [W_qkv[:, (s * n_heads + h) * DH : (s * n_heads + h) * DH + DH]
                for s in range(3) for h in hs]
        wqkv_c = np.ascontiguousarray(np.concatenate(cols, axis=1))
        wout_c = np.ascontiguousarray(
            W_out[hs[0] * DH : (hs[-1] + 1) * DH, :]
        )
        in_maps.append(
            {
                "xT": xT,
                "wqkv": wqkv_c,
                "wout": wout_c,
                "cosT": cosT,
                "ssinT": ssinT,
                "rot": rot,
                "colmask": colmask,
                "rqT": rqT,
                "dvalB": dvalB,
                "dvalT": dvalT,
                "mqT": mqT,
                "cmask128": cmask128,
            }
        )
    return in_maps


_CACHED_NC = None


def assemble(results, t=T, d=D, n_cores=N_CORES, rs_chunks=8):
    """Reassemble per-core ReduceScatter slices into the full output."""
    rows_per_chunk = t // rs_chunks
    rows_per_core = rows_per_chunk // n_cores
    out = np.empty((t, d), dtype=BF16)
    for c in range(n_cores):
        oc = np.asarray(results[c]["out"])
        if oc.dtype != BF16:
            oc = oc.view(BF16)
        for ci in range(rs_chunks):
            lo = ci * rows_per_chunk + c * rows_per_core
            out[lo : lo + rows_per_core] = oc[
                ci * rows_per_core : (ci + 1) * rows_per_core
            ]
    return out


def kernel(x, W_qkv, W_out, cos, sin, mask):
    """Full inputs in, full output out. Shards across 8 NeuronCores."""
    global _CACHED_NC
    from concourse import bass_utils

    if _CACHED_NC is None:
        _CACHED_NC = build_nc()
    nc = _CACHED_NC

    in_maps = prepare_in_maps(x, W_qkv, W_out, cos, sin, mask)
    res = bass_utils.run_bass_kernel_spmd(
        nc, in_maps, core_ids=list(range(N_CORES))
    )
    return assemble(res.results)


# revision 35
# speedup vs baseline: 1.7712x; 1.0194x over previous
"""Distributed Trainium2 attention kernel (8 NeuronCores, head tensor-parallel).

Reference semantics (T=4096, D=2048, H=16, DH=128):
  qkv = bf16(x @ W_qkv); q,k,v per head; RoPE(split-half) on q,k;
  mask = ((m_q & m_k) | eye) & causal; softmax(q k^T / sqrt(DH) masked);
  out = bf16((probs @ v) @ W_out)

Sharding: head tensor-parallel. Core c owns heads (2c, 2c+1): W_qkv column
shard, W_out row shard, full x (replicated, passed pre-transposed).
Each core computes its heads' SDPA, its out-projection partial, then a
chunked ReduceScatter sums partials; core c emits output rows
[chunk*1024 + c*128 : +128) for each of the 4 chunks. Host reassembles.

Device-side layout choices:
  - x passed as xT [D, T] so the D contraction dim is the partition dim.
  - q,k computed weight-stationary -> born transposed [DH, T]; v
    transposed back to natural [T, DH] via PE (PV lhsT layout).
  - RoPE: partition-rotate by 64 via a permutation-matrix matmul on PE,
    sign folded into a host-precomputed ssinT table; combine on DVE.
  - SDPA in transposed-scores form: scoresT[k, q] tiles over 512-query
    quads; exp (no max-subtraction; scores are O(5) here) evacuates the
    scores psum straight into the PV rhs -- no probs transposes.
  - key padding mask folded into the exp bias (per-k = per-partition);
    within-block causal via one precomputed 0/1 [128,128] multiply.
  - softmax denominators via a ones-column matmul, transposed to
    q-partition layout with 4 tiny K=1 matmuls; normalization deferred
    to the out-projection epilogue (per-partition scalars there), with
    a mid-accumulation ratio scale to handle the two heads' different
    denominators in one psum group.
  - masked queries (attend only self) fixed by blending v^T via colmask
    zeroing + (1-m) add; all blend scalars stay per-partition.
"""

import os
import sys

import numpy as np

sys.path.insert(0, "/opt/trn_rl_repo")

import ml_dtypes

BF16 = ml_dtypes.bfloat16

# problem constants (hardcoded per harness contract)
T, D, H, DH = 4096, 2048, 16, 128
N_CORES = 8
ROPE_BASE = 10000.0


def _rs_chunk_sizes(qb_n, rs_chunks):
    """Reduce-scatter chunk sizes in q-blocks: front-loaded so the final
    collective (pure exposed tail) is tiny."""
    if qb_n == 32:
        return [6, 6, 6, 5, 4, 2, 2, 1]
    per = qb_n // rs_chunks
    return [per] * rs_chunks


def build_nc(
    t=T,
    d=D,
    n_cores=N_CORES,
    hl=H // N_CORES,  # heads per core
    kch=512,  # scores k-chunk (free dim of scores matmul)
    tch=512,  # qkv t-chunk
    rs_chunks=8,  # reduce-scatter chunks
):
    import concourse.bass as bass
    import concourse.mybir as mybir
    import concourse.tile as tile
    from concourse import bacc
    from concourse.masks import make_identity

    f32 = mybir.dt.float32
    bf16 = mybir.dt.bfloat16

    P = 128
    kd = d // P  # contraction chunks for qkv
    qb_n = t // P  # q-blocks of 128 rows
    nt = t // tch  # t-chunks in qkv phase
    jl = hl * P  # local out-proj contraction width
    chunk_sizes = _rs_chunk_sizes(qb_n, rs_chunks)
    chunk_starts = [0]
    for cs_ in chunk_sizes:
        chunk_starts.append(chunk_starts[-1] + cs_)
    qb_to_chunk = {}
    for ci_, cs_ in enumerate(chunk_sizes):
        for ri_ in range(cs_):
            qb_to_chunk[chunk_starts[ci_] + ri_] = (ci_, ri_)
    t_out = t // n_cores  # output rows per core
    scale = 1.0 / np.sqrt(DH)

    nc = bacc.Bacc(
        "TRN2", target_bir_lowering=False, debug=False, num_devices=n_cores
    )

    xT = nc.dram_tensor("xT", [d, t], bf16, kind="ExternalInput").ap()
    wqkv = nc.dram_tensor("wqkv", [d, 3 * jl], bf16, kind="ExternalInput").ap()
    wout_d = nc.dram_tensor("wout", [jl, d], bf16, kind="ExternalInput").ap()
    cosT_d = nc.dram_tensor("cosT", [P, t], f32, kind="ExternalInput").ap()
    ssinT_d = nc.dram_tensor("ssinT", [P, t], f32, kind="ExternalInput").ap()
    # colmask: mask[k] as 0/1 bf16, pre-broadcast to all 128 partitions
    colmask_d = nc.dram_tensor("colmask", [P, t], bf16, kind="ExternalInput").ap()
    # rqT[p, qb] = 0 if mask[qb*128+p] else -1e9 (folded into exp bias)
    rqT_d = nc.dram_tensor("rqT", [P, qb_n], f32, kind="ExternalInput").ap()
    # dvalB[p, q] = 1 - mask[q], broadcast to all partitions
    dvalB_d = nc.dram_tensor("dvalB", [P, t], bf16, kind="ExternalInput").ap()
    # dvalT[p, qb] = 1 - mask[qb*128+p] (q-partition layout)
    dvalT_d = nc.dram_tensor("dvalT", [P, qb_n], f32, kind="ExternalInput").ap()
    # mqT[p, qb] = mask[qb*128+p] (q-partition layout)
    mqT_d = nc.dram_tensor("mqT", [P, qb_n], f32, kind="ExternalInput").ap()
    # cmask128[p, j] = 1 if j >= p else 0 (within-block causal, T-orientation)
    cmask128_d = nc.dram_tensor("cmask128", [P, P], bf16, kind="ExternalInput").ap()
    out_d = nc.dram_tensor("out", [t_out, d], bf16, kind="ExternalOutput").ap()

    with tile.TileContext(nc) as tc:
        with tc.tile_pool(name="persist", bufs=1) as persist:
            # persistent SBUF tensors
            ident = persist.tile([P, P], bf16, name="ident")
            make_identity(nc, ident)
            wq_sb = persist.tile([P, kd, 3 * hl, P], bf16, name="wq_sb")
            wqkv_r = wqkv.rearrange("(kd p) (c j) -> p kd c j", p=P, j=P)
            for k in range(kd):
                nc.sync.dma_start(wq_sb[:, k], wqkv_r[:, k])
            wout_sb = persist.tile([P, hl, d], bf16, name="wout_sb")
            nc.sync.dma_start(wout_sb, wout_d.rearrange("(h p) x -> p h x", p=P))

            # per-head persistent activations
            qT = [persist.tile([P, t], bf16, name=f"qT{h}") for h in range(hl)]
            kT = [persist.tile([P, t], bf16, name=f"kT{h}") for h in range(hl)]
            vT = [persist.tile([P, t], bf16, name=f"vT{h}") for h in range(hl)]
            v_nat = [
                persist.tile([P, qb_n, P], bf16, name=f"vnat{h}") for h in range(hl)
            ]
            oT = [persist.tile([P, t], bf16, name=f"oT{h}") for h in range(hl)]

            # ---------------- phase 1: qkv + rope + v transpose ----------
            with (
                tc.tile_pool(name="ph1", bufs=2) as ph1,
                tc.tile_pool(name="cs", bufs=1) as cspool,
                tc.tile_pool(name="ps_qkv", bufs=1, space="PSUM") as ps_qkv,
                tc.tile_pool(name="ps_aux", bufs=2, space="PSUM") as ps_aux,
            ):
                cosT_sb = cspool.tile([P, t], f32, name="cosT_sb")
                nc.sync.dma_start(cosT_sb, cosT_d)
                ssinT_sb = cspool.tile([P, t], f32, name="ssinT_sb")
                nc.sync.dma_start(ssinT_sb, ssinT_d)

                for tc_i in range(nt):
                    tsl = slice(tc_i * tch, (tc_i + 1) * tch)
                    xt = ph1.tile([P, kd, tch], bf16, tag="xt")
                    xT_r = xT.rearrange("(kd p) x -> p kd x", p=P)
                    for k in range(kd):
                        nc.sync.dma_start(xt[:, k], xT_r[:, k, tsl])
                    for c in range(3 * hl):  # q0,q1,k0,k1,v0,v1
                        ps = ps_qkv.tile([P, tch], mybir.dt.float32, tag=f"ps{c}")
                        for k in range(kd):
                            nc.tensor.matmul(
                                ps,
                                lhsT=wq_sb[:, k, c],
                                rhs=xt[:, k],
                                start=(k == 0),
                                stop=(k == kd - 1),
                            )
                        if c < 2 * hl:  # q or k: cast, rotate, rope-combine
                            dst = qT[c] if c < hl else kT[c - hl]
                            qbf = ph1.tile([P, tch], bf16, tag="qbf")
                            nc.scalar.copy(qbf, ps)
                            # rotate-half: partition shift by 64 via two
                            # SBUF->SBUF DMAs (keeps PE free)
                            shift = ph1.tile([P, tch], bf16, tag="shift")
                            nc.sync.dma_start(shift[0:64], qbf[64:128])
                            nc.sync.dma_start(shift[64:128], qbf[0:64])
                            t1 = ph1.tile([P, tch], f32, tag="t1")
                            nc.vector.tensor_tensor(
                                t1, qbf, cosT_sb[:, tsl], mybir.AluOpType.mult
                            )
                            t2 = ph1.tile([P, tch], f32, tag="t2")
                            nc.vector.tensor_tensor(
                                t2, shift, ssinT_sb[:, tsl], mybir.AluOpType.mult
                            )
                            nc.vector.tensor_tensor(
                                dst[:, tsl], t1, t2, mybir.AluOpType.add
                            )
                        else:  # v: just cast
                            nc.scalar.copy(vT[c - 2 * hl][:, tsl], ps)

                # v: [DH, T] -> natural [T-block, DH] tiles
                for h in range(hl):
                    for b in range(qb_n):
                        pst = ps_aux.tile([P, P], bf16, tag="aux")
                        nc.tensor.transpose(
                            pst, vT[h][:, b * P : (b + 1) * P], ident
                        )
                        nc.scalar.copy(v_nat[h][:, b], pst)

            # ---------------- phase 2: SDPA + out-proj + RS --------------
            # Transposed-scores formulation: scoresT[k, q] tiles per 128-k
            # block over a 512-query "quad"; exp evacuates psum straight to
            # the PV rhs; denominator via a ones-column matmul; softmax
            # normalization + masked-row fixup fused into the single oT
            # evacuation (per-query scalars partition-broadcast on gpsimd).
            qw = 512  # queries per quad
            with (
                tc.tile_pool(name="ph2", bufs=3) as ph2,
                tc.tile_pool(name="pt", bufs=3) as ptpool,
                tc.tile_pool(name="msk", bufs=1) as mskpool,
                tc.tile_pool(name="dram", bufs=1, space="DRAM") as dram,
                tc.tile_pool(name="ps_s", bufs=2, space="PSUM") as ps_s,
                tc.tile_pool(name="ps_o", bufs=1, space="PSUM") as ps_o,
                tc.tile_pool(name="ps_d", bufs=2, space="PSUM") as ps_d,
                tc.tile_pool(name="ps_dt", bufs=1, space="PSUM") as ps_dt,
                tc.tile_pool(name="ps_out", bufs=2, space="PSUM") as ps_out,
            ):
                colmask_sb = mskpool.tile([P, t], bf16, name="colmask_sb")
                nc.sync.dma_start(colmask_sb, colmask_d)
                rqT_sb = mskpool.tile([P, qb_n], f32, name="rqT_sb")
                nc.sync.dma_start(rqT_sb, rqT_d)
                dvalB_sb = mskpool.tile([P, t], bf16, name="dvalB_sb")
                nc.sync.dma_start(dvalB_sb, dvalB_d)
                cm128_sb = mskpool.tile([P, P], bf16, name="cm128_sb")
                nc.sync.dma_start(cm128_sb, cmask128_d)
                ones_sb = mskpool.tile([P, 1], bf16, name="ones_sb")
                nc.vector.memset(ones_sb, 1.0)
                dvalT_sb = mskpool.tile([P, qb_n], f32, name="dvalT_sb")
                nc.sync.dma_start(dvalT_sb, dvalT_d)
                mqT_sb = mskpool.tile([P, qb_n], f32, name="mqT_sb")
                nc.sync.dma_start(mqT_sb, mqT_d)
                rs_in = [
                    dram.tile([cs_ * P, d], bf16, name=f"rs_in{ci}")
                    for ci, cs_ in enumerate(chunk_sizes)
                ]
                rs_out = [
                    dram.tile([cs_ * P // n_cores, d], bf16, name=f"rs_out{ci}")
                    for ci, cs_ in enumerate(chunk_sizes)
                ]

                n_quads = t // qw
                qb_per_quad = qw // P  # 4
                for g in range(n_quads):
                    gsl = slice(g * qw, (g + 1) * qw)
                    nsk = (g + 1) * qb_per_quad  # causal k-blocks for quad
                    dsum_bfs = []
                    invs = {}
                    for h in range(hl):
                        pso = ps_o.tile([P, qw], f32, tag="oT")
                        psd = ps_d.tile([1, qw], f32, tag="den")

                        def emit_score(sk, h=h, g=g):
                            br = sk - g * qb_per_quad  # >=0 in diag region
                            lo = br * P if br >= 0 else 0
                            psT = ps_s.tile([P, qw], f32, tag="scT",
                                            name="psT")
                            nc.tensor.matmul(
                                psT[:, lo:],
                                lhsT=kT[h][:, sk * P : (sk + 1) * P],
                                rhs=qT[h][:, g * qw + lo : (g + 1) * qw],
                                start=True,
                                stop=True,
                            )
                            pT = ptpool.tile([P, qw], bf16, tag="pT",
                                             name="pT")
                            # exp; per-k padding mask folded into the bias
                            nc.scalar.activation(
                                pT[:, lo:],
                                psT[:, lo:],
                                mybir.ActivationFunctionType.Exp,
                                scale=float(scale),
                                bias=rqT_sb[:, sk : sk + 1],
                            )
                            if br >= 0:
                                # within-block causal on the partial 128 cols
                                nc.vector.tensor_tensor(
                                    pT[:, lo : lo + P],
                                    pT[:, lo : lo + P],
                                    cm128_sb,
                                    mybir.AluOpType.mult,
                                )
                            return pT, lo

                        def emit_pv(sk, pT, lo, h=h, pso=pso, psd=psd,
                                    nsk=nsk):
                            nc.tensor.matmul(
                                pso[:, lo:],
                                lhsT=v_nat[h][:, sk],
                                rhs=pT[:, lo:],
                                start=(sk == 0),
                                stop=(sk == nsk - 1),
                            )
                            nc.tensor.matmul(
                                psd[:, lo:],
                                lhsT=ones_sb,
                                rhs=pT[:, lo:],
                                start=(sk == 0),
                                stop=(sk == nsk - 1),
                            )

                        # software-pipelined emission (lookahead 2) so the
                        # PE stream never stalls on exp: scT(sk+1), scT(sk+2)
                        # run while exp(sk) finishes, then PV(sk)
                        LA = 2
                        stage = {}
                        for sk in range(nsk):
                            stage[sk] = emit_score(sk)
                            if sk - LA >= 0:
                                emit_pv(sk - LA, *stage.pop(sk - LA))
                        for sk in range(max(0, nsk - LA), nsk):
                            emit_pv(sk, *stage.pop(sk))
                        # oT kept UNNORMALIZED (bf16 is scale-free); masked-q
                        # garbage zeroed via colmask; masked queries attend
                        # only themselves -> blend in v^T * (1-m[q])
                        nc.vector.tensor_tensor(
                            oT[h][:, gsl], pso, colmask_sb[:, gsl],
                            mybir.AluOpType.mult,
                        )
                        vblend = ph2.tile([P, qw], bf16, tag="vblend")
                        nc.gpsimd.tensor_tensor(
                            vblend, vT[h][:, gsl], dvalB_sb[:, gsl],
                            mybir.AluOpType.mult,
                        )
                        nc.vector.tensor_tensor(
                            oT[h][:, gsl], oT[h][:, gsl], vblend,
                            mybir.AluOpType.add,
                        )
                        # denominators -> q-partition layout: transpose the
                        # [1, 512] row into [128, 4] via 4 tiny K=1 matmuls,
                        # so the reciprocal runs on all 128 lanes
                        dsum_bf = ph2.tile([1, qw], bf16, tag=f"dsum{h}")
                        nc.vector.tensor_copy(dsum_bf, psd)
                        denT = ps_dt.tile([P, qb_per_quad], f32, tag="denT")
                        for j in range(qb_per_quad):
                            nc.tensor.matmul(
                                denT[:, j : j + 1],
                                lhsT=dsum_bf[0:1, j * P : (j + 1) * P],
                                rhs=ones_sb[0:1, 0:1],
                                start=True,
                                stop=True,
                            )
                        gq = slice(g * qb_per_quad, (g + 1) * qb_per_quad)
                        rec = ph2.tile([P, qb_per_quad], f32, tag=f"rec{h}")
                        nc.vector.reciprocal(rec, denT)
                        # inv3 = m[q]/denom + (1-m[q])  (masked rows pass the
                        # blended v through unscaled)
                        inv3 = ph2.tile([P, qb_per_quad], f32, tag=f"inv3{h}")
                        nc.vector.tensor_tensor(
                            inv3, rec, mqT_sb[:, gq], mybir.AluOpType.mult
                        )
                        nc.vector.tensor_tensor(
                            inv3, inv3, dvalT_sb[:, gq], mybir.AluOpType.add
                        )
                        if h == hl - 1 and hl > 1:
                            # jnv = m*denom + (1-m); r = inv3_0 * jnv_1 so a
                            # mid-accumulation psum scale by r followed by a
                            # final scale by inv3_1 yields per-head softmax
                            # normalization inside one accumulation group
                            jnv = ph2.tile([P, qb_per_quad], f32, tag="jnv")
                            nc.vector.tensor_tensor(
                                jnv, denT, mqT_sb[:, gq], mybir.AluOpType.mult
                            )
                            nc.vector.tensor_tensor(
                                jnv, jnv, dvalT_sb[:, gq], mybir.AluOpType.add
                            )
                            rsc = ph2.tile([P, qb_per_quad], f32, tag="rsc")
                            nc.vector.tensor_tensor(
                                rsc, inv3_prev, jnv, mybir.AluOpType.mult
                            )
                        inv3_prev = inv3

                    # out-projection + RS for the quad's 4 q-blocks
                    for qq in range(qb_per_quad):
                        qb = g * qb_per_quad + qq
                        qsl = slice(qb * P, (qb + 1) * P)
                        partial = ph2.tile([P, d], bf16, tag="partial")
                        for ntile in range(d // 512):
                            nsl = slice(ntile * 512, (ntile + 1) * 512)
                            pso2 = ps_out.tile([P, 512], f32, tag="outp")
                            for h in range(hl):
                                nc.tensor.matmul(
                                    pso2,
                                    lhsT=oT[h][:, qsl],
                                    rhs=wout_sb[:, h, nsl],
                                    start=(h == 0),
                                    stop=True,
                                    skip_group_check=(h > 0),
                                )
                                if h < hl - 1:
                                    nc.vector.tensor_scalar_mul(
                                        pso2, pso2, rsc[:, qq : qq + 1]
                                    )
                            nc.vector.tensor_scalar_mul(
                                partial[:, nsl], pso2, inv3[:, qq : qq + 1]
                            )
                        ci, ri = qb_to_chunk[qb]
                        nc.sync.dma_start(
                            rs_in[ci][ri * P : (ri + 1) * P, :], partial
                        )
                        if ri == chunk_sizes[ci] - 1:
                            nc.gpsimd.collective_compute(
                                "ReduceScatter",
                                mybir.AluOpType.add,
                                replica_groups=[list(range(n_cores))],
                                ins=[rs_in[ci].opt()],
                                outs=[rs_out[ci].opt()],
                            )
                            rows = chunk_sizes[ci] * P // n_cores
                            orow = chunk_starts[ci] * P // n_cores
                            nc.sync.dma_start(
                                out_d[orow : orow + rows, :],
                                rs_out[ci],
                            )

    nc.compile()
    return nc


def prepare_in_maps(x, W_qkv, W_out, cos, sin, mask, n_cores=N_CORES, hl=H // N_CORES):
    """Host-side sharding. Returns list of per-core input dicts."""
    t, d = x.shape
    x = np.asarray(x, dtype=BF16)
    W_qkv = np.asarray(W_qkv, dtype=BF16)
    W_out = np.asarray(W_out, dtype=BF16)
    cos = np.asarray(cos, dtype=np.float32)
    sin = np.asarray(sin, dtype=np.float32)
    m = np.asarray(mask, dtype=bool)

    xT = np.ascontiguousarray(x.T)
    cosT = np.ascontiguousarray(cos.T)
    sign = np.where(np.arange(DH) < DH // 2, -1.0, 1.0).astype(np.float32)
    ssinT = np.ascontiguousarray(sin.T * sign[:, None])

    mf = m.astype(np.float32)
    colmask = np.ascontiguousarray(
        np.broadcast_to(mf.astype(BF16)[None, :], (DH, t))
    )
    rqT = np.ascontiguousarray(
        np.where(mf, np.float32(0.0), np.float32(-1e9)).reshape(-1, DH).T
    )
    dvalB = np.ascontiguousarray(
        np.broadcast_to((1.0 - mf).astype(BF16)[None, :], (DH, t))
    )
    dvalT = np.ascontiguousarray((1.0 - mf).astype(np.float32).reshape(-1, DH).T)
    mqT = np.ascontiguousarray(mf.astype(np.float32).reshape(-1, DH).T)
    cmask128 = (np.arange(DH)[None, :] >= np.arange(DH)[:, None]).astype(BF16)

    n_heads = W_qkv.shape[1] // 3 // DH
    in_maps = []
    for c in range(n_cores):
        hs = [c * hl + i for i in range(hl)]
        cols = [W_qkv[:, (s * n_heads + h) * DH : (s * n_heads + h) * DH + DH]
                for s in range(3) for h in hs]
        wqkv_c = np.ascontiguousarray(np.concatenate(cols, axis=1))
        wout_c = np.ascontiguousarray(
            W_out[hs[0] * DH : (hs[-1] + 1) * DH, :]
        )
        in_maps.append(
            {
                "xT": xT,
                "wqkv": wqkv_c,
                "wout": wout_c,
                "cosT": cosT,
                "ssinT": ssinT,
                "colmask": colmask,
                "rqT": rqT,
                "dvalB": dvalB,
                "dvalT": dvalT,
                "mqT": mqT,
                "cmask128": cmask128,
            }
        )
    return in_maps


_CACHED_NC = None


def assemble(results, t=T, d=D, n_cores=N_CORES, rs_chunks=8):
    """Reassemble per-core ReduceScatter slices into the full output."""
    P = 128
    qb_n = t // P
    chunk_sizes = _rs_chunk_sizes(qb_n, rs_chunks)
    out = np.empty((t, d), dtype=BF16)
    for c in range(n_cores):
        oc = np.asarray(results[c]["out"])
        if oc.dtype != BF16:
            oc = oc.view(BF16)
        row0 = 0  # chunk start in global rows
        orow = 0  # chunk start in per-core output rows
        for cs_ in chunk_sizes:
            rows = cs_ * P // n_cores
            lo = row0 + c * rows
            out[lo : lo + rows] = oc[orow : orow + rows]
            row0 += cs_ * P
            orow += rows
    return out


def kernel(x, W_qkv, W_out, cos, sin, mask):
    """Full inputs in, full output out. Shards across 8 NeuronCores."""
    global _CACHED_NC
    from concourse import bass_utils

    if _CACHED_NC is None:
        _CACHED_NC = build_nc()
    nc = _CACHED_NC

    in_maps = prepare_in_maps(x, W_qkv, W_out, cos, sin, mask)
    res = bass_utils.run_bass_kernel_spmd(
        nc, in_maps, core_ids=list(range(N_CORES))
    )
    return assemble(res.results)
